# revision 8
# baseline (speedup 1.0000x reference)
"""Trainium2 Bass kernel for nn_CrossAttentionFusionGated.

Contract: kernel(**inputs) takes FULL unsharded inputs (as produced by
setup_inputs()) and returns the full output tuple (fused, attn_weights,
g_used). Internally shards data-parallel over batch B=8 across the 8
NeuronCores (one batch element per core) and runs a single SPMD Bass
program via run_bass_kernel_spmd.

Algorithm notes (algebraic fusion, validated in numpy against the jax
reference):
  Cross-attention only attends to SPK=4 speaker tokens, so the q and
  output projections fold into tiny per-batch matrices:
    scores^T = M^T @ X^T / 8,  M = Wq^T @ Kbd            [512, 32]
    attn     = w_g @ Vo,       Vo = Vbd^T @ Wo^T + 1*bo/8 [32, 512]
  (the bo/8 fold works because each head's softmax row sums to 1 and
  there are 8 heads). The gate LN folds as
    z  = X @ (gamma_g*Wg1)^T - mu ⊗ a,   a = rowsum(gamma_g*Wg1)
    g2 = rstd * (Wg2 @ relu(z)) + bg2    (when beta_g/bg1 == 0)
  The residual add (X + w_g@Vo) is done on the PE via an
  identity-matmul accumulation into the same PSUM bank.
"""

import numpy as np

import concourse.bass as bass
import concourse.bacc as bacc
import concourse.tile as tile
from concourse import mybir
from concourse.bass_utils import run_bass_kernel_spmd

F32 = mybir.dt.float32
AF = mybir.ActivationFunctionType
ALU = mybir.AluOpType

B, T, D = 8, 4096, 512
H, DH, SPK, HG = 8, 64, 4, 128
HS = H * SPK  # 32
EPS = 1e-5
TB = 512           # tokens per block
NBLK = T // TB     # 8
NSUB = TB // 128   # 4


def _nz(x):
    return not np.all(np.asarray(x) == 0)


def build_program(flags):
    """Emit the SPMD program for one core (one batch element).

    flags: dict of bools for which bias/affine inputs are nonzero.
    """
    nc = bacc.Bacc()

    # ---- DRAM I/O (per core) ----
    x_d = nc.dram_tensor("x", [T, D], F32, kind="ExternalInput")
    se_d = nc.dram_tensor("se", [D], F32, kind="ExternalInput")
    WeT_d = nc.dram_tensor("WeT", [D, SPK * D], F32, kind="ExternalInput")
    WkT_d = nc.dram_tensor("WkT", [D, D], F32, kind="ExternalInput")
    WvT_d = nc.dram_tensor("WvT", [D, D], F32, kind="ExternalInput")
    Wq_d = nc.dram_tensor("Wq", [D, D], F32, kind="ExternalInput")
    WoT_d = nc.dram_tensor("WoT", [D, D], F32, kind="ExternalInput")
    Wg1T_d = nc.dram_tensor("Wg1T", [D, HG], F32, kind="ExternalInput")
    Wg2_d = nc.dram_tensor("Wg2", [1, HG], F32, kind="ExternalInput")
    posT_d = nc.dram_tensor("posT", [D, SPK], F32, kind="ExternalInput")
    eye_d = nc.dram_tensor("eye128", [128, 128], F32, kind="ExternalInput")
    S32_d = nc.dram_tensor("S32", [HS, H], F32, kind="ExternalInput")
    S8_d = nc.dram_tensor("S8", [H, HS], F32, kind="ExternalInput")
    P8_d = nc.dram_tensor("P8", [HS, SPK], F32, kind="ExternalInput")
    ones_d = nc.dram_tensor("onesc", [128, 1], F32, kind="ExternalInput")

    opt_in = {}
    if flags["be"]:
        opt_in["beT"] = nc.dram_tensor("beT", [SPK * D], F32, kind="ExternalInput")
    if flags["bq"]:
        opt_in["bqT"] = nc.dram_tensor("bqT", [D], F32, kind="ExternalInput")
    if flags["bk"]:
        opt_in["bkT"] = nc.dram_tensor("bkT", [D], F32, kind="ExternalInput")
    if flags["bv"]:
        opt_in["bvT"] = nc.dram_tensor("bvT", [D], F32, kind="ExternalInput")
    if flags["bo"]:
        opt_in["bo_r"] = nc.dram_tensor("bo_r", [1, D], F32, kind="ExternalInput")
        opt_in["o8"] = nc.dram_tensor("o8", [1, HS], F32, kind="ExternalInput")
    if flags["gg"]:
        opt_in["ggT"] = nc.dram_tensor("ggT", [D], F32, kind="ExternalInput")
    if flags["gb_c1"]:  # gln_beta or bg1 nonzero
        opt_in["gbT"] = nc.dram_tensor("gbT", [D], F32, kind="ExternalInput")
        opt_in["bg1T"] = nc.dram_tensor("bg1T", [HG], F32, kind="ExternalInput")
    if flags["lg"]:
        opt_in["lgam"] = nc.dram_tensor("lgam", [1, D], F32, kind="ExternalInput")
    if flags["lb"]:
        opt_in["lbet"] = nc.dram_tensor("lbet", [1, D], F32, kind="ExternalInput")

    of_d = nc.dram_tensor("out_fused", [T, D], F32, kind="ExternalOutput")
    ow_d = nc.dram_tensor("out_aw", [T, SPK], F32, kind="ExternalOutput")
    og_d = nc.dram_tensor("out_g", [T, 1], F32, kind="ExternalOutput")

    bg2_val = float(flags["bg2_val"])

    with tile.TileContext(nc) as tc:
        with (
            tc.tile_pool(name="consts", bufs=1) as consts,
            tc.tile_pool(name="prep_w", bufs=3) as prep_w,     # streamed weights
            tc.tile_pool(name="prep_sb", bufs=1) as prep_sb,   # long-lived prep results
            tc.tile_pool(name="xin", bufs=8) as xin,
            tc.tile_pool(name="xts", bufs=6) as xts,
            tc.tile_pool(name="loop_sb", bufs=2) as loop_sb,
            tc.tile_pool(name="small_sb", bufs=4) as small_sb,
            tc.tile_pool(name="out_sb", bufs=4) as outp,
            tc.tile_pool(name="ps_big", bufs=2, space="PSUM") as ps_big,
            tc.tile_pool(name="ps_fp", bufs=2, space="PSUM") as ps_fp,
            tc.tile_pool(name="ps_sm", bufs=4, space="PSUM") as ps_sm,
        ):
            # ================= constants =================
            eye = consts.tile([128, 128], F32, tag="eye")
            nc.sync.dma_start(out=eye, in_=eye_d[:, :])
            S32 = consts.tile([HS, H], F32, tag="S32")
            nc.sync.dma_start(out=S32, in_=S32_d[:, :])
            S8 = consts.tile([H, HS], F32, tag="S8")
            nc.sync.dma_start(out=S8, in_=S8_d[:, :])
            P8 = consts.tile([HS, SPK], F32, tag="P8")
            nc.sync.dma_start(out=P8, in_=P8_d[:, :])
            onesc = consts.tile([128, 1], F32, tag="onesc")
            nc.sync.dma_start(out=onesc, in_=ones_d[:, :])
            epsc = consts.tile([128, 1], F32, tag="epsc")
            nc.vector.memset(epsc, EPS)

            # ================= prep: speaker tokens =================
            # seT [128, 4] : column c = se[c*128:(c+1)*128]
            seT = prep_sb.tile([128, 4], F32, tag="seT")
            nc.sync.dma_start(out=seT, in_=se_d[:].rearrange("(c p) -> p c", p=128))

            # e_row [1, 2048] = se @ We^T  (via lhsT = seT chunks, rhs = WeT tiles)
            e_sb = prep_sb.tile([1, SPK * D], F32, tag="e_sb")
            for eb in range(4):  # e column blocks of 512
                e_ps = ps_sm.tile([1, 512], F32, tag="smps")
                for dc in range(4):
                    wet = prep_w.tile([128, 512], F32, tag="wet")
                    nc.sync.dma_start(
                        out=wet, in_=WeT_d[dc * 128:(dc + 1) * 128, eb * 512:(eb + 1) * 512])
                    nc.tensor.matmul(e_ps, lhsT=seT[:, dc:dc + 1],
                                     rhs=wet, start=(dc == 0), stop=(dc == 3))
                if flags["be"]:
                    ebt = small_sb.tile([1, 512], F32, tag="ebt")
                    nc.sync.dma_start(out=ebt, in_=opt_in["beT"][eb * 512:(eb + 1) * 512]
                                      .rearrange("(one f) -> one f", one=1))
                    nc.vector.tensor_add(out=e_sb[:, eb * 512:(eb + 1) * 512],
                                         in0=e_ps, in1=ebt)
                    nc.scalar.activation(out=e_sb[:, eb * 512:(eb + 1) * 512],
                                         in_=e_sb[:, eb * 512:(eb + 1) * 512], func=AF.Relu)
                else:
                    nc.scalar.activation(out=e_sb[:, eb * 512:(eb + 1) * 512],
                                         in_=e_ps, func=AF.Relu)

            # spkT [128, 4] per d-chunk: transpose e rows + spk_pos
            posT = prep_sb.tile([128, 4, 4], F32, tag="posT")  # [128, dc, s]
            nc.sync.dma_start(out=posT,
                              in_=posT_d[:, :].rearrange("(c p) s -> p c s", p=128))
            spkT = []
            for dc in range(4):
                sp_ps = ps_sm.tile([128, 4], F32, tag="smps")
                for s in range(SPK):
                    nc.tensor.transpose(
                        sp_ps[:, s:s + 1],
                        e_sb[0:1, s * D + dc * 128: s * D + (dc + 1) * 128],
                        eye[0:1, 0:1])
                st = prep_sb.tile([128, 4], F32, tag=f"spkT{dc}")
                nc.vector.tensor_add(out=st, in0=sp_ps, in1=posT[:, dc, :])
                spkT.append(st)

            # kT / vT [128, 4] per o-chunk, then block-diag Kbd / Vbd [128, 32]
            def proj_bd(WT_d, bias_key, tagp):
                bd = []
                for oc in range(4):
                    pj_ps = ps_sm.tile([128, 4], F32, tag="smps")
                    for dc in range(4):
                        wt = prep_w.tile([128, 128], F32, tag="wkv")
                        nc.sync.dma_start(
                            out=wt,
                            in_=WT_d[dc * 128:(dc + 1) * 128, oc * 128:(oc + 1) * 128])
                        nc.tensor.matmul(pj_ps, lhsT=wt, rhs=spkT[dc],
                                         start=(dc == 0), stop=(dc == 3))
                    if bias_key is not None:
                        bt = small_sb.tile([128, 1], F32, tag="bkv")
                        nc.sync.dma_start(
                            out=bt, in_=opt_in[bias_key][oc * 128:(oc + 1) * 128]
                            .rearrange("(p one) -> p one", one=1))
                        nc.vector.tensor_scalar(out=pj_ps, in0=pj_ps, scalar1=bt,
                                                scalar2=None, op0=ALU.add)
                    bdt = prep_sb.tile([128, HS], F32, tag=f"{tagp}{oc}")
                    nc.vector.memset(bdt, 0.0)
                    h0 = 2 * oc
                    nc.scalar.activation(out=bdt[0:64, h0 * SPK:(h0 + 1) * SPK],
                                         in_=pj_ps[0:64, :], func=AF.Copy)
                    nc.scalar.activation(out=bdt[64:128, (h0 + 1) * SPK:(h0 + 2) * SPK],
                                         in_=pj_ps[64:128, :], func=AF.Copy)
                    bd.append(bdt)
                return bd

            Kbd = proj_bd(WkT_d, "bkT" if flags["bk"] else None, "kbd")
            Vbd = proj_bd(WvT_d, "bvT" if flags["bv"] else None, "vbd")

            # M [128, 32] per d-chunk = Wq^T @ Kbd
            Wq_sb = []
            for oc in range(4):
                wq = prep_sb.tile([128, 512], F32, tag=f"wq{oc}")
                nc.sync.dma_start(out=wq, in_=Wq_d[oc * 128:(oc + 1) * 128, :])
                Wq_sb.append(wq)
            M_sb = []
            for dc in range(4):
                m_ps = ps_sm.tile([128, HS], F32, tag="smps")
                for oc in range(4):
                    nc.tensor.matmul(m_ps, lhsT=Wq_sb[oc][:, dc * 128:(dc + 1) * 128],
                                     rhs=Kbd[oc], start=(oc == 0), stop=(oc == 3))
                mt = prep_sb.tile([128, HS], F32, tag=f"M{dc}")
                nc.scalar.activation(out=mt, in_=m_ps, func=AF.Copy)
                M_sb.append(mt)

            # score bias c [32, 1] = Kbd^T @ bq / 8 (optional)
            c_sb = None
            if flags["bq"]:
                bqT = prep_sb.tile([128, 4], F32, tag="bqT")
                nc.sync.dma_start(out=bqT,
                                  in_=opt_in["bqT"][:].rearrange("(c p) -> p c", p=128))
                c_ps = ps_sm.tile([HS, 1], F32, tag="smps")
                for oc in range(4):
                    nc.tensor.matmul(c_ps, lhsT=Kbd[oc], rhs=bqT[:, oc:oc + 1],
                                     start=(oc == 0), stop=(oc == 3))
                c_sb = prep_sb.tile([HS, 1], F32, tag="c_sb")
                nc.scalar.activation(out=c_sb, in_=c_ps, func=AF.Copy, scale=0.125)

            # Vo [32, 512] = Vbd^T @ Wo^T (+ bo/8 rank-1 fold)
            vo_ps = ps_big.tile([HS, 512], F32, tag="bigps")
            for dc in range(4):
                wot = prep_w.tile([128, 512], F32, tag="wot")
                nc.sync.dma_start(out=wot, in_=WoT_d[dc * 128:(dc + 1) * 128, :])
                nc.tensor.matmul(vo_ps, lhsT=Vbd[dc], rhs=wot,
                                 start=(dc == 0), stop=not flags["bo"] and dc == 3)
            if flags["bo"]:
                o8 = small_sb.tile([1, HS], F32, tag="o8")
                nc.sync.dma_start(out=o8, in_=opt_in["o8"][:, :])
                bor = small_sb.tile([1, D], F32, tag="bor")
                nc.sync.dma_start(out=bor, in_=opt_in["bo_r"][:, :])
                nc.tensor.matmul(vo_ps, lhsT=o8, rhs=bor, start=False, stop=True)
            Vo_sb = prep_sb.tile([HS, 512], F32, tag="Vo")
            nc.scalar.activation(out=Vo_sb, in_=vo_ps, func=AF.Copy)

            # gate weights: Wg1p^T tiles [128, 128] (optionally gamma_g-scaled)
            Wg1pT = []
            for dc in range(4):
                wg1 = prep_sb.tile([128, HG], F32, tag=f"wg1_{dc}")
                nc.sync.dma_start(out=wg1, in_=Wg1T_d[dc * 128:(dc + 1) * 128, :])
                Wg1pT.append(wg1)
            if flags["gg"]:
                ggT = prep_sb.tile([128, 4], F32, tag="ggT")
                nc.sync.dma_start(out=ggT,
                                  in_=opt_in["ggT"][:].rearrange("(c p) -> p c", p=128))
                for dc in range(4):
                    nc.vector.tensor_scalar(out=Wg1pT[dc], in0=Wg1pT[dc],
                                            scalar1=ggT[:, dc:dc + 1], scalar2=None,
                                            op0=ALU.mult)
            # neg_a [1, 128] = -rowsum(Wg1p)
            a_ps = ps_sm.tile([1, HG], F32, tag="smps")
            for dc in range(4):
                nc.tensor.matmul(a_ps, lhsT=onesc, rhs=Wg1pT[dc],
                                 start=(dc == 0), stop=(dc == 3))
            neg_a = prep_sb.tile([1, HG], F32, tag="neg_a")
            nc.scalar.activation(out=neg_a, in_=a_ps, func=AF.Copy, scale=-1.0)

            # c1 [128, 1] = Wg1 @ gln_beta + bg1 (optional, general path)
            c1_sb = None
            if flags["gb_c1"]:
                gbT = prep_sb.tile([128, 4], F32, tag="gbT")
                nc.sync.dma_start(out=gbT,
                                  in_=opt_in["gbT"][:].rearrange("(c p) -> p c", p=128))
                c1_ps = ps_sm.tile([HG, 1], F32, tag="smps")
                for dc in range(4):
                    nc.tensor.matmul(c1_ps, lhsT=Wg1pT[dc], rhs=gbT[:, dc:dc + 1],
                                     start=(dc == 0), stop=(dc == 3))
                bg1t = small_sb.tile([HG, 1], F32, tag="bg1t")
                nc.sync.dma_start(out=bg1t, in_=opt_in["bg1T"][:].rearrange("(p one) -> p one", one=1))
                c1_sb = prep_sb.tile([HG, 1], F32, tag="c1")
                nc.vector.tensor_add(out=c1_sb, in0=c1_ps, in1=bg1t)

            wg2T = prep_sb.tile([HG, 1], F32, tag="wg2T")
            nc.sync.dma_start(out=wg2T, in_=Wg2_d[0, :].rearrange("(p one) -> p one", one=1))

            ln_g = ln_b = None
            if flags["lg"]:
                ln_g = consts.tile([128, D], F32, tag="ln_g")
                nc.sync.dma_start(out=ln_g, in_=opt_in["lgam"].to_broadcast([128, D]))
            if flags["lb"]:
                ln_b = consts.tile([128, D], F32, tag="ln_b")
                nc.sync.dma_start(out=ln_b, in_=opt_in["lbet"].to_broadcast([128, D]))

            # ================= main loop =================
            for blk in range(NBLK):
                t0 = blk * TB
                # load X tiles
                X = []
                for i in range(NSUB):
                    xi = xin.tile([128, D], F32, tag="x")
                    nc.sync.dma_start(out=xi, in_=x_d[t0 + i * 128: t0 + (i + 1) * 128, :])
                    X.append(xi)
                # transpose X -> XT chunks [128(d), 512(t)]
                XT = []
                for dc in range(4):
                    xt = xts.tile([128, TB], F32, tag="xt")
                    for i in range(NSUB):
                        tp = ps_sm.tile([128, 128], F32, tag="smps")
                        nc.tensor.transpose(tp, X[i][:, dc * 128:(dc + 1) * 128], eye)
                        nc.any.tensor_copy(out=xt[:, i * 128:(i + 1) * 128], in_=tp)
                    XT.append(xt)

                # ---- scores / softmax ----
                sc_ps = ps_big.tile([HS, TB], F32, tag="bigps")
                for dc in range(4):
                    nc.tensor.matmul(sc_ps, lhsT=M_sb[dc], rhs=XT[dc],
                                     start=(dc == 0), stop=(dc == 3))
                exp_sb = loop_sb.tile([HS, TB], F32, tag="exp")
                if c_sb is not None:
                    nc.scalar.activation(out=exp_sb, in_=sc_ps, func=AF.Exp,
                                         scale=0.125, bias=c_sb)
                else:
                    nc.scalar.activation(out=exp_sb, in_=sc_ps, func=AF.Exp, scale=0.125)
                dn_ps = ps_sm.tile([H, TB], F32, tag="smps")
                nc.tensor.matmul(dn_ps, lhsT=S32, rhs=exp_sb, start=True, stop=True)
                recip = loop_sb.tile([H, TB], F32, tag="recip")
                nc.vector.reciprocal(out=recip, in_=dn_ps)
                rd_ps = ps_sm.tile([HS, TB], F32, tag="smps")
                nc.tensor.matmul(rd_ps, lhsT=S8, rhs=recip, start=True, stop=True)
                w_sb = loop_sb.tile([HS, TB], F32, tag="w")
                nc.vector.tensor_mul(out=w_sb, in0=exp_sb, in1=rd_ps)

                # ---- attn_weights out ----
                aw_sb = outp.tile([128, NSUB, SPK], F32, tag="aw")
                for i in range(NSUB):
                    awp = ps_sm.tile([128, SPK], F32, tag="smps")
                    nc.tensor.matmul(awp, lhsT=w_sb[:, i * 128:(i + 1) * 128], rhs=P8,
                                     start=True, stop=True)
                    nc.scalar.activation(out=aw_sb[:, i, :], in_=awp, func=AF.Copy)
                nc.sync.dma_start(
                    out=ow_d[t0:t0 + TB, :].rearrange("(i p) s -> p i s", p=128),
                    in_=aw_sb)

                # ---- gate ----
                z_ps = ps_big.tile([HG, TB], F32, tag="bigps")
                for dc in range(4):
                    nc.tensor.matmul(z_ps, lhsT=Wg1pT[dc], rhs=XT[dc],
                                     start=(dc == 0), stop=False)
                # gate-LN stats of X: per-tsub var via bn_stats, mu row via PE
                mvx = []
                for i in range(NSUB):
                    st6 = small_sb.tile([128, 6], F32, tag="bnst")
                    nc.vector.bn_stats(out=st6, in_=X[i])
                    mv = small_sb.tile([128, 2], F32, tag="mv")
                    nc.vector.bn_aggr(out=mv, in_=st6)
                    mvx.append(mv)
                mu_ps = ps_sm.tile([1, TB], F32, tag="smps")
                for dc in range(4):
                    nc.tensor.matmul(mu_ps, lhsT=onesc, rhs=XT[dc],
                                     start=(dc == 0), stop=(dc == 3))
                mu_row = small_sb.tile([1, TB], F32, tag="mu_row")
                nc.scalar.activation(out=mu_row, in_=mu_ps, func=AF.Copy,
                                     scale=1.0 / D)
                statT = None
                if c1_sb is not None:
                    statT = small_sb.tile([2, TB], F32, tag="statT")
                    for i in range(NSUB):
                        mvt = ps_sm.tile([2, 128], F32, tag="smps")
                        nc.tensor.transpose(mvt, mvx[i], eye)
                        nc.scalar.activation(out=statT[:, i * 128:(i + 1) * 128],
                                             in_=mvt, func=AF.Copy)
                # z += (-a) x mu
                nc.tensor.matmul(z_ps, lhsT=neg_a, rhs=mu_row,
                                 start=False, stop=True)
                g1T = loop_sb.tile([HG, TB], F32, tag="g1T")
                rstdv = None
                if c1_sb is None:
                    nc.scalar.activation(out=g1T, in_=z_ps, func=AF.Relu)
                else:
                    # general path: g1 = relu(rstd*z + c1); rstd broadcast via PE
                    varv = small_sb.tile([1, TB], F32, tag="varv")
                    nc.scalar.activation(out=varv, in_=statT[1:2, :], func=AF.Sqrt,
                                         bias=epsc[0:1, :])
                    rstdv = small_sb.tile([1, TB], F32, tag="rstdv")
                    nc.vector.reciprocal(out=rstdv, in_=varv)
                    rb_ps = ps_sm.tile([HG, TB], F32, tag="smps")
                    nc.tensor.matmul(rb_ps, lhsT=onesc[0:1, :].to_broadcast([1, HG]),
                                     rhs=rstdv, start=True, stop=True)
                    zr = loop_sb.tile([HG, TB], F32, tag="zr")
                    nc.vector.tensor_mul(out=zr, in0=z_ps, in1=rb_ps)
                    nc.scalar.activation(out=g1T, in_=zr, func=AF.Relu, bias=c1_sb)

                # g2 per tsub + sigmoid; collect g into [128, NSUB]
                g4 = small_sb.tile([128, NSUB], F32, tag="g4")
                gbc_ps = ps_big.tile([HS, TB], F32, tag="bigps")
                for i in range(NSUB):
                    g2p = ps_sm.tile([128, 1], F32, tag="smps")
                    nc.tensor.matmul(g2p, lhsT=g1T[:, i * 128:(i + 1) * 128], rhs=wg2T,
                                     start=True, stop=True)
                    y = small_sb.tile([128, 1], F32, tag="y")
                    if c1_sb is None:
                        sd = small_sb.tile([128, 1], F32, tag="sd")
                        nc.scalar.activation(out=sd, in_=mvx[i][:, 1:2], func=AF.Sqrt,
                                             bias=epsc)
                        rstd_g = small_sb.tile([128, 1], F32, tag="rstd_g")
                        nc.vector.reciprocal(out=rstd_g, in_=sd)
                        nc.vector.tensor_mul(out=y, in0=g2p, in1=rstd_g)
                    else:
                        nc.scalar.activation(out=y, in_=g2p, func=AF.Copy)
                    if bg2_val != 0.0:
                        nc.vector.tensor_scalar(out=y, in0=y, scalar1=bg2_val,
                                                scalar2=None, op0=ALU.add)
                    # sigmoid via exp+reciprocal (ACT Sigmoid table less accurate)
                    ey = small_sb.tile([128, 1], F32, tag="ey")
                    nc.scalar.activation(out=ey, in_=y, func=AF.Exp, scale=-1.0)
                    nc.vector.tensor_scalar(out=ey, in0=ey, scalar1=1.0,
                                            scalar2=None, op0=ALU.add)
                    nc.vector.reciprocal(out=g4[:, i:i + 1], in_=ey)
                    # broadcast g across 32 partitions: lhsT = g (free-bcast), rhs = eye
                    nc.tensor.matmul(
                        gbc_ps[:, i * 128:(i + 1) * 128],
                        lhsT=g4[:, i:i + 1].to_broadcast([128, HS]),
                        rhs=eye, start=True, stop=True)
                nc.sync.dma_start(
                    out=og_d[t0:t0 + TB, :].rearrange("(i p) one -> p (i one)", p=128),
                    in_=g4)

                wg_sb = loop_sb.tile([HS, TB], F32, tag="wg")
                nc.vector.tensor_mul(out=wg_sb, in0=w_sb, in1=gbc_ps)

                # ---- attn + residual + final LN ----
                for i in range(NSUB):
                    fp = ps_fp.tile([128, D], F32, tag="fp")
                    nc.tensor.matmul(fp, lhsT=wg_sb[:, i * 128:(i + 1) * 128],
                                     rhs=Vo_sb, start=True, stop=False)
                    nc.tensor.matmul(fp, lhsT=eye, rhs=X[i], start=False, stop=True)
                    st6 = small_sb.tile([128, 6], F32, tag="bnst")
                    nc.vector.bn_stats(out=st6, in_=fp)
                    mvf = small_sb.tile([128, 2], F32, tag="mvf")
                    nc.vector.bn_aggr(out=mvf, in_=st6)
                    sdf = small_sb.tile([128, 1], F32, tag="sdf")
                    nc.scalar.activation(out=sdf, in_=mvf[:, 1:2], func=AF.Sqrt,
                                         bias=epsc)
                    rstd_f = small_sb.tile([128, 1], F32, tag="rstd_f")
                    nc.vector.reciprocal(out=rstd_f, in_=sdf)
                    nmr = small_sb.tile([128, 1], F32, tag="nmr")
                    nc.vector.tensor_scalar(out=nmr, in0=mvf[:, 0:1], scalar1=rstd_f,
                                            scalar2=-1.0, op0=ALU.mult, op1=ALU.mult)
                    ot = outp.tile([128, D], F32, tag="ot")
                    nc.scalar.activation(out=ot, in_=fp, func=AF.Identity,
                                         scale=rstd_f, bias=nmr)
                    if ln_g is not None:
                        nc.vector.tensor_mul(out=ot, in0=ot, in1=ln_g)
                    if ln_b is not None:
                        nc.vector.tensor_add(out=ot, in0=ot, in1=ln_b)
                    nc.sync.dma_start(out=of_d[t0 + i * 128: t0 + (i + 1) * 128, :],
                                      in_=ot)
    nc.finalize()
    return nc


_cache = {}


def kernel(token_emb, speaker_emb, We, be, spk_pos, Wq, bq, Wk, bk, Wv, bv,
           Wo, bo, gln_gamma, gln_beta, Wg1, bg1, Wg2, bg2, ln_gamma, ln_beta):
    f32 = lambda a: np.ascontiguousarray(np.asarray(a, np.float32))
    token_emb, speaker_emb = f32(token_emb), f32(speaker_emb)
    We, be, spk_pos = f32(We), f32(be), f32(spk_pos)
    Wq, bq, Wk, bk, Wv, bv = f32(Wq), f32(bq), f32(Wk), f32(bk), f32(Wv), f32(bv)
    Wo, bo = f32(Wo), f32(bo)
    gln_gamma, gln_beta, Wg1, bg1 = f32(gln_gamma), f32(gln_beta), f32(Wg1), f32(bg1)
    Wg2, bg2, ln_gamma, ln_beta = f32(Wg2), f32(bg2), f32(ln_gamma), f32(ln_beta)

    flags = {
        "be": _nz(be), "bq": _nz(bq), "bk": _nz(bk), "bv": _nz(bv), "bo": _nz(bo),
        "gg": not np.all(gln_gamma == 1.0),
        "gb_c1": _nz(gln_beta) or _nz(bg1),
        "lg": not np.all(ln_gamma == 1.0), "lb": _nz(ln_beta),
        "bg2_val": float(np.asarray(bg2).reshape(-1)[0]),
    }
    key = tuple(sorted((k, bool(v) if k != "bg2_val" else v != 0.0)
                       for k, v in flags.items()))
    if key not in _cache:
        _cache[key] = build_program(flags)
    nc = _cache[key]

    # shared (replicated) tensors
    shared = {
        "WeT": f32(We.T), "WkT": f32(Wk.T), "WvT": f32(Wv.T), "Wq": Wq,
        "WoT": f32(Wo.T), "Wg1T": f32(Wg1.T), "Wg2": Wg2,
        "posT": f32(spk_pos[0].T),
        "eye128": np.eye(128, dtype=np.float32),
        "S32": f32(np.kron(np.eye(H), np.ones((SPK, 1)))),
        "S8": f32(np.kron(np.eye(H), np.ones((1, SPK)))),
        "P8": f32(np.kron(np.ones((H, 1)) / H, np.eye(SPK))),
        "onesc": np.ones((128, 1), np.float32),
    }
    if flags["be"]:
        shared["beT"] = be
    if flags["bq"]:
        shared["bqT"] = bq
    if flags["bk"]:
        shared["bkT"] = bk
    if flags["bv"]:
        shared["bvT"] = bv
    if flags["bo"]:
        shared["bo_r"] = f32(bo[None, :])
        shared["o8"] = np.full((1, HS), 0.125, np.float32)
    if flags["gg"]:
        shared["ggT"] = gln_gamma
    if flags["gb_c1"]:
        shared["gbT"] = gln_beta
        shared["bg1T"] = bg1
    if flags["lg"]:
        shared["lgam"] = f32(ln_gamma[None, :])
    if flags["lb"]:
        shared["lbet"] = f32(ln_beta[None, :])

    in_maps = []
    for b in range(B):
        m = dict(shared)
        m["x"] = f32(token_emb[b])
        m["se"] = f32(speaker_emb[b])
        in_maps.append(m)

    res = run_bass_kernel_spmd(nc, in_maps, core_ids=list(range(B)))
    fused = np.stack([res.results[b]["out_fused"] for b in range(B)])
    aw = np.stack([res.results[b]["out_aw"] for b in range(B)])
    gu = np.stack([res.results[b]["out_g"] for b in range(B)])
    return fused, aw, gu
